# revision 1
# baseline (speedup 1.0000x reference)
"""CAFE-interpolation kernel for 8 Trainium2 NeuronCores.

Strategy: shard the T axis (1024 = 8 x 128) across cores. Every core holds a
T-slice of ALL 128 samples, so the sr[partner_idx] gather is core-local.

Math: with mask_b = (im_b > thr_b) in {0,1}^D and c_b = is_dominant_b*(1-m_b):

  out[b] = x[b] + c_b * ( mask[p_b] . x[p_b] - mask[b] . x[b] )
         = x[b] + c_b * ((P - I) @ (mask . x))[b]

so the whole mixup collapses into one constant-permutation matmul over the
sample axis plus elementwise ops:

  stage 1: im_partial[b, d] = sum_{t in slice} grad[b,t,d]*x[b,t,d]
           All on DVE: elementwise mul + strided free-axis reduce over t
           (samples live on partitions), accumulate across t-groups,
           scale by 1/1024 at the end.
  AllReduce im_partial [128, 512] across the 8 cores (~256 KB).
  stage 2: exact 52nd/53rd largest of each im row: iterative max-extraction
           with fused mask-out+reduce (tensor_scalar + tensor_tensor_reduce),
           thr = v459 + 0.9*(v460-v459) exactly like jnp.quantile,
           mask = im > thr; cvec = is_dominant*(1-mixup).
  stage 3: per t-pair: xm = x[:,t,:] * mask        (DVE / GpSimd alternating)
                       q  = (P-I)^T @ xm           (PE, constant weights)
                       out = (q * cvec) + x[:,t,:] (fused scalar_tensor_tensor)

The same program works for every (partner_idx, is_dominant): the metadata
enters only through the pmi/dom input tensors, so it compiles once per
process.
"""

import os
import numpy as np

B, T, D = 128, 1024, 512
N_CORES = 8
T_LOC = T // N_CORES  # 128
KTOP = 53  # need the 52nd and 53rd largest of each 512-row
TG1 = 8  # t-steps per stage-1 group
TG3 = 2  # t-steps per stage-3 group

_CACHE: dict = {}
LAST_RESULT = None


def _build():
    import concourse.mybir as mybir
    import concourse.tile as tile
    from concourse import bacc

    f32 = mybir.dt.float32
    Alu = mybir.AluOpType
    AX = mybir.AxisListType

    _dbg = os.environ.get("KBUILD_DEBUG") == "1"

    nc = bacc.Bacc(
        "TRN2", target_bir_lowering=False, debug=False, num_devices=N_CORES
    )
    x_sl = nc.dram_tensor("x_sl", [B, T_LOC, D], f32, kind="ExternalInput")
    g_sl = nc.dram_tensor("g_sl", [B, T_LOC, D], f32, kind="ExternalInput")
    m_in = nc.dram_tensor("m_in", [B, 1], f32, kind="ExternalInput")
    dom_in = nc.dram_tensor("dom_in", [B, 1], f32, kind="ExternalInput")
    pmi_in = nc.dram_tensor("pmi_in", [B, B], f32, kind="ExternalInput")
    out_sl = nc.dram_tensor("out_sl", [B, T_LOC, D], f32, kind="ExternalOutput")
    if _dbg:
        dbg_im = nc.dram_tensor("dbg_im", [B, D], f32, kind="ExternalOutput")
        dbg_mask = nc.dram_tensor("dbg_mask", [B, D], f32, kind="ExternalOutput")

    with tile.TileContext(nc) as tc:
        with tc.tile_pool(name="persist", bufs=1) as pp:
            m_t = pp.tile([B, 1], f32)
            nc.sync.dma_start(m_t[:], m_in[:])
            dom_t = pp.tile([B, 1], f32)
            nc.sync.dma_start(dom_t[:], dom_in[:])
            pmi_t = pp.tile([B, B], f32)
            nc.sync.dma_start(pmi_t[:], pmi_in[:])
            im_all = pp.tile([B, D], f32)
            cur_a = pp.tile([B, D], f32)
            cur_b = pp.tile([B, D], f32)
            mv = pp.tile([B, 64], f32)
            mask = pp.tile([B, D], f32)
            cvec = pp.tile([B, 1], f32)
            imacc = pp.tile([B, D], f32)

            # ---- stage 1: im_partial = sum_t x*g on DVE ----
            with (
                tc.tile_pool(name="ld1", bufs=2) as ld1,
                tc.tile_pool(name="pr1", bufs=2) as pr1,
                tc.tile_pool(name="ccp", bufs=1, space="DRAM") as ccp,
            ):
                n_g1 = T_LOC // TG1
                for i in range(n_g1):
                    t0 = i * TG1
                    xt = ld1.tile([B, TG1, D], f32, tag="x1")
                    gt = ld1.tile([B, TG1, D], f32, tag="g1")
                    nc.sync.dma_start(xt[:], x_sl[:, t0 : t0 + TG1, :])
                    nc.sync.dma_start(gt[:], g_sl[:, t0 : t0 + TG1, :])
                    prod = pr1.tile([B, TG1, D], f32, tag="prod")
                    nc.vector.tensor_tensor(prod[:], xt[:], gt[:], op=Alu.mult)
                    # contiguous pairwise tree-sum over t (the strided-innermost
                    # tensor_reduce measures ~1.6x slower than dense adds)
                    f4 = pr1.tile([B, TG1 // 2, D], f32, tag="f4")
                    nc.vector.tensor_tensor(
                        f4[:], prod[:, 0 : TG1 // 2, :], prod[:, TG1 // 2 :, :],
                        op=Alu.add,
                    )
                    f2 = pr1.tile([B, TG1 // 4, D], f32, tag="f2")
                    nc.vector.tensor_tensor(
                        f2[:], f4[:, 0 : TG1 // 4, :], f4[:, TG1 // 4 :, :],
                        op=Alu.add,
                    )
                    if i == 0:
                        nc.vector.tensor_tensor(
                            imacc[:], f2[:, 0, :], f2[:, 1, :], op=Alu.add
                        )
                    else:
                        part = pr1.tile([B, D], f32, tag="part")
                        nc.vector.tensor_tensor(
                            part[:], f2[:, 0, :], f2[:, 1, :], op=Alu.add
                        )
                        nc.vector.tensor_tensor(
                            imacc[:], imacc[:], part[:], op=Alu.add
                        )
                # scale by 1/T (exact power of two)
                nc.vector.tensor_scalar(
                    imacc[:], imacc[:], scalar1=1.0 / T, scalar2=None, op0=Alu.mult
                )

                # ---- AllReduce the partial importance ----
                cc_in_t = ccp.tile([B, D], f32, name="cc_in_t")
                cc_out_t = ccp.tile([B, D], f32, name="cc_out_t")
                nc.gpsimd.dma_start(cc_in_t[:], imacc[:])
                nc.gpsimd.collective_compute(
                    "AllReduce",
                    Alu.add,
                    replica_groups=[list(range(N_CORES))],
                    ins=[cc_in_t.opt()],
                    outs=[cc_out_t.opt()],
                )
                nc.gpsimd.dma_start(im_all[:], cc_out_t[:])

            # ---- stage 2: exact top-52/53 values per row ----
            with (
                tc.tile_pool(name="sel", bufs=2) as selp,
                tc.tile_pool(name="psumw", bufs=1, space="PSUM") as psumw,
            ):
                # Iterative exact max-extraction. Removed elements become 0,
                # which is a safe sentinel because the top-53 of a 512-wide
                # zero-mean row are positive (P(not) ~ 1e-90 for randn data);
                # surviving values are untouched (exact order statistics).
                cur, nxt = im_all, cur_b
                nc.vector.reduce_max(mv[:, 0:1], cur[:], axis=AX.X)
                for k in range(1, KTOP):
                    # cur' = (cur < m_{k-1}) * cur ; mv[k] = max(cur')
                    nc.vector.scalar_tensor_tensor(
                        nxt[:],
                        cur[:],
                        mv[:, k - 1 : k],
                        cur[:],
                        op0=Alu.is_lt,
                        op1=Alu.mult,
                    )
                    nc.vector.reduce_max(mv[:, k : k + 1], nxt[:], axis=AX.X)
                    cur = nxt
                    nxt = cur_a if cur is cur_b else cur_b

                # PE warm-up during the selection window (junk results)
                qw = psumw.tile([B, D], f32)
                for _ in range(20):
                    nc.tensor.matmul(
                        qw[:], pmi_t[:], im_all[:], start=True, stop=True
                    )

                # thr = v459 + 0.9*(v460 - v459); v460 = mv[:,51], v459 = mv[:,52]
                dl = pp.tile([B, 1], f32)
                nc.vector.tensor_tensor(
                    dl[:], mv[:, 51:52], mv[:, 52:53], op=Alu.subtract
                )
                dl9 = pp.tile([B, 1], f32)
                nc.vector.tensor_scalar(
                    dl9[:], dl[:], scalar1=0.9, scalar2=None, op0=Alu.mult
                )
                thr_t = pp.tile([B, 1], f32)
                nc.vector.tensor_tensor(thr_t[:], mv[:, 52:53], dl9[:], op=Alu.add)

                mask_src = im_all
                nc.vector.tensor_scalar(
                    mask[:],
                    mask_src[:],
                    scalar1=thr_t[:, 0:1],
                    scalar2=None,
                    op0=Alu.is_gt,
                )

                # cvec = dom * (1 - m)
                om_t = pp.tile([B, 1], f32)
                nc.vector.tensor_scalar(
                    om_t[:],
                    m_t[:],
                    scalar1=-1.0,
                    scalar2=1.0,
                    op0=Alu.mult,
                    op1=Alu.add,
                )
                nc.vector.tensor_tensor(cvec[:], om_t[:], dom_t[:], op=Alu.mult)

                if _dbg:
                    nc.gpsimd.dma_start(dbg_im[:], im_all[:])
                    nc.gpsimd.dma_start(dbg_mask[:], mask[:])

            # ---- stage 3: out = x + c * ((P-I) @ (mask.x)) ----
            with (
                tc.tile_pool(name="x3", bufs=36) as x3p,
                tc.tile_pool(name="t3", bufs=4) as t3p,
                tc.tile_pool(name="psumq", bufs=3, space="PSUM") as psumq,
            ):
                for gi, t0 in enumerate(range(0, T_LOC, TG3)):
                    xt3 = x3p.tile([B, TG3, D], f32, tag="x3t")
                    nc.sync.dma_start(xt3[:], x_sl[:, t0 : t0 + TG3, :])
                    q = psumq.tile([B, TG3, D], f32, tag="q")
                    ot = t3p.tile([B, TG3, D], f32, tag="ot")
                    # one wide mask-multiply for the whole t-pair; mask is
                    # broadcast over t by a zero-stride middle AP dim
                    xm = t3p.tile([B, TG3, D], f32, tag="xm")
                    eng = nc.vector if gi % 2 == 0 else nc.gpsimd
                    for j in range(TG3):
                        eng.tensor_tensor(
                            xm[:, j, :], xt3[:, j, :], mask[:], op=Alu.mult
                        )
                    for j in range(TG3):
                        nc.tensor.matmul(
                            q[:, j, :], pmi_t[:], xm[:, j, :], start=True, stop=True
                        )
                    # out = (q * c) + x over the whole t-pair at once
                    nc.vector.scalar_tensor_tensor(
                        ot[:],
                        q[:],
                        cvec[:, 0:1],
                        xt3[:],
                        op0=Alu.mult,
                        op1=Alu.add,
                    )
                    nc.scalar.dma_start(out_sl[:, t0 : t0 + TG3, :], ot[:])
    nc.compile()
    return nc


def _build_copy():
    """All-non-dominant fast path: output == x."""
    import concourse.mybir as mybir
    import concourse.tile as tile
    from concourse import bacc

    f32 = mybir.dt.float32
    nc = bacc.Bacc(
        "TRN2", target_bir_lowering=False, debug=False, num_devices=N_CORES
    )
    x_sl = nc.dram_tensor("x_sl", [B, T_LOC, D], f32, kind="ExternalInput")
    nc.dram_tensor("g_sl", [B, T_LOC, D], f32, kind="ExternalInput")
    nc.dram_tensor("m_in", [B, 1], f32, kind="ExternalInput")
    nc.dram_tensor("dom_in", [B, 1], f32, kind="ExternalInput")
    nc.dram_tensor("pmi_in", [B, B], f32, kind="ExternalInput")
    out_sl = nc.dram_tensor("out_sl", [B, T_LOC, D], f32, kind="ExternalOutput")
    with tile.TileContext(nc):
        CG = 8
        for i, b0 in enumerate(range(0, B, CG)):
            eng = nc.sync if i % 2 == 0 else nc.scalar
            eng.dma_start(out_sl[b0 : b0 + CG], x_sl[b0 : b0 + CG])
    nc.compile()
    return nc


def kernel(x, scenario_gradient, mixup_strength, scenario, partner_idx, is_dominant):
    global LAST_RESULT
    from concourse.bass_utils import run_bass_kernel_spmd

    x = np.ascontiguousarray(np.asarray(x, dtype=np.float32))
    g = np.ascontiguousarray(np.asarray(scenario_gradient, dtype=np.float32))
    m = np.asarray(mixup_strength, dtype=np.float32).reshape(B, 1)
    p = np.asarray(partner_idx, dtype=np.int64).ravel()
    dm = np.asarray(is_dominant, dtype=bool).ravel()

    any_dom = bool(dm.any())
    key = "main" if any_dom else "copy"
    nc = _CACHE.get(key)
    if nc is None:
        nc = _build() if any_dom else _build_copy()
        _CACHE[key] = nc

    dom_f = dm.astype(np.float32).reshape(B, 1)
    p_eff = np.where(dm, p, np.arange(B, dtype=np.int64))
    # pmi = (P - I)^T with P[b, p_b] = 1: pmi[k, b] = [k == p_b] - [k == b]
    pmi = np.zeros((B, B), dtype=np.float32)
    pmi[p_eff, np.arange(B)] += 1.0
    pmi[np.arange(B), np.arange(B)] -= 1.0

    in_maps = []
    for c in range(N_CORES):
        sl = slice(c * T_LOC, (c + 1) * T_LOC)
        in_maps.append(
            {
                "x_sl": np.ascontiguousarray(x[:, sl, :]),
                "g_sl": np.ascontiguousarray(g[:, sl, :]),
                "m_in": m,
                "dom_in": dom_f,
                "pmi_in": pmi,
            }
        )

    res = run_bass_kernel_spmd(nc, in_maps, core_ids=list(range(N_CORES)))
    LAST_RESULT = res

    out = np.empty((B, T, D), dtype=np.float32)
    for c in range(N_CORES):
        out[:, c * T_LOC : (c + 1) * T_LOC, :] = res.results[c]["out_sl"]
    return out



# revision 4
# speedup vs baseline: 1.1028x; 1.1028x over previous
"""CAFE-interpolation kernel for 8 Trainium2 NeuronCores.

Strategy: shard the T axis (1024 = 8 x 128) across cores. Every core holds a
T-slice of ALL 128 samples, so the sr[partner_idx] gather is core-local.

Math: with mask_b = (im_b > thr_b) in {0,1}^D and c_b = is_dominant_b*(1-m_b):

  out[b] = x[b] + c_b * ( mask[p_b] . x[p_b] - mask[b] . x[b] )

Only dominant rows differ from x, so the device returns just those rows
(packed via the matmul's stationary gather matrix); the host assembles
out = x.copy() and scatters the device rows in.

Per-core pipeline (inputs are fp16, host-converted; halves read traffic and
enables the DVE 2x 16-bit mode):

  stage 1: im_partial[b,d] = sum_{t in slice} g*x. DVE: fp16 product +
           pairwise tree-add (fp16), f32 accumulation across t-groups on
           GpSimd. x tiles stay resident in SBUF for stage 3 (16 MB).
  AllReduce im_partial [128, 512] f32 across 8 cores (~256 KB).
  stage 2: exact 52nd/53rd largest per row via 7 rounds of the DVE max-8
           instruction + match_replace (top-k extraction, 8 ranks/round);
           thr = v459 + 0.9*(v460-v459) exactly like jnp.quantile (the
           1/T mean scale cancels: mask is scale-invariant).
  stage 3: per t-group: xm = x * mask (DVE, fp16); PSUM accumulates
           A^T@x + Pc^T@xm where A packs dominant rows and Pc = c*(P - I);
           the f32 PSUM tile IS the output -> DMA straight to DRAM.

The same program works for every (partner_idx, is_dominant, mixup): the
metadata enters only through the amat/pmat input tensors; compile is keyed
only on n_dom.
"""

import os
import numpy as np

B, T, D = 128, 1024, 512
N_CORES = 8
T_LOC = T // N_CORES  # 128
TG1 = 8  # stage-1 t-steps per group (16 groups)
TG3 = 4  # stage-3 t-steps per group (32 groups); [n_dom, 4*512] f32 = 4 PSUM banks
NSEL = 7  # max-8 rounds: ranks 1..56 cover v460 (rank 52) and v459 (rank 53)

_CACHE: dict = {}
LAST_RESULT = None


def _build(n_dom: int):
    import concourse.mybir as mybir
    import concourse.tile as tile
    from concourse import bacc

    f32 = mybir.dt.float32
    f16 = mybir.dt.float16
    Alu = mybir.AluOpType

    _dbg = os.environ.get("KBUILD_DEBUG") == "1"

    nc = bacc.Bacc(
        "TRN2", target_bir_lowering=False, debug=False, num_devices=N_CORES
    )
    x_sl = nc.dram_tensor("x_sl", [B, T_LOC, D], f16, kind="ExternalInput")
    g_sl = nc.dram_tensor("g_sl", [B, T_LOC, D], f16, kind="ExternalInput")
    amat_in = nc.dram_tensor("amat", [B, n_dom], f16, kind="ExternalInput")
    pmat_in = nc.dram_tensor("pmat", [B, n_dom], f16, kind="ExternalInput")
    out_sl = nc.dram_tensor("out_sl", [n_dom, T_LOC, D], f16, kind="ExternalOutput")
    if _dbg:
        dbg_im = nc.dram_tensor("dbg_im", [B, D], f32, kind="ExternalOutput")
        dbg_mask = nc.dram_tensor("dbg_mask", [B, D], f32, kind="ExternalOutput")

    n_g1 = T_LOC // TG1

    with tile.TileContext(nc) as tc:
        with tc.tile_pool(name="persist", bufs=1) as pp:
            amat_t = pp.tile([B, n_dom], f16)
            nc.sync.dma_start(amat_t[:], amat_in[:])
            pmat_t = pp.tile([B, n_dom], f16)
            nc.sync.dma_start(pmat_t[:], pmat_in[:])

            # persistent x cache: 16 tiles of [128, 8, 512] fp16 (16 MB)
            xts = [pp.tile([B, TG1, D], f16, name=f"xc{i}") for i in range(n_g1)]

            imacc = pp.tile([B, D], f32)
            im_all = pp.tile([B, D], f32)
            sel_a = pp.tile([B, D], f32)
            sel_b = pp.tile([B, D], f32)
            mv = pp.tile([B, 8 * NSEL], f32)
            mask3 = pp.tile([B, 1, D], f16)
            thr_t = pp.tile([B, 1], f32)
            d1 = pp.tile([B, 1], f32)

            # ---- stage 1: im_partial = sum_t x*g ----
            with (
                tc.tile_pool(name="gld", bufs=2) as gld,
                tc.tile_pool(name="wk1", bufs=2) as wk1,
                tc.tile_pool(name="ccp", bufs=1, space="DRAM") as ccp,
            ):
                for i in range(n_g1):
                    t0 = i * TG1
                    nc.sync.dma_start(xts[i][:], x_sl[:, t0 : t0 + TG1, :])
                    gt = gld.tile([B, TG1, D], f16, tag="g1")
                    nc.sync.dma_start(gt[:], g_sl[:, t0 : t0 + TG1, :])
                    prod = wk1.tile([B, TG1, D], f16, tag="prod")
                    nc.vector.tensor_tensor(prod[:], xts[i][:], gt[:], op=Alu.mult)
                    l1 = wk1.tile([B, TG1 // 2, D], f16, tag="l1")
                    nc.vector.tensor_tensor(
                        l1[:], prod[:, 0 : TG1 // 2, :], prod[:, TG1 // 2 :, :],
                        op=Alu.add,
                    )
                    l2 = wk1.tile([B, TG1 // 4, D], f16, tag="l2")
                    nc.vector.tensor_tensor(
                        l2[:], l1[:, 0 : TG1 // 4, :], l1[:, TG1 // 4 :, :],
                        op=Alu.add,
                    )
                    if i == 0:
                        # f32 accumulator seeded directly on DVE
                        nc.vector.tensor_tensor(
                            imacc[:], l2[:, 0, :], l2[:, 1, :], op=Alu.add
                        )
                    else:
                        l3 = wk1.tile([B, D], f16, tag="l3")
                        nc.vector.tensor_tensor(
                            l3[:], l2[:, 0, :], l2[:, 1, :], op=Alu.add
                        )
                        # accumulate on GpSimd to keep DVE free
                        nc.gpsimd.tensor_tensor(
                            imacc[:], imacc[:], l3[:], op=Alu.add
                        )

                # ---- AllReduce the partial importance (no 1/T scale:
                # quantile mask is scale-invariant) ----
                cc_in_t = ccp.tile([B, D], f32, name="cc_in_t")
                cc_out_t = ccp.tile([B, D], f32, name="cc_out_t")
                nc.gpsimd.dma_start(cc_in_t[:], imacc[:])
                nc.gpsimd.collective_compute(
                    "AllReduce",
                    Alu.add,
                    replica_groups=[list(range(N_CORES))],
                    ins=[cc_in_t.opt()],
                    outs=[cc_out_t.opt()],
                )
                nc.gpsimd.dma_start(im_all[:], cc_out_t[:])

            # ---- stage 2: ranks 52/53 via 8-wide max extraction ----
            with tc.tile_pool(name="psumw", bufs=1, space="PSUM") as psumw:
                cur = im_all
                for r in range(NSEL):
                    nc.vector.max(mv[:, 8 * r : 8 * r + 8], cur[:])
                    if r < NSEL - 1:
                        nxt = sel_a if r % 2 == 0 else sel_b
                        nc.vector.match_replace(
                            nxt[:], mv[:, 8 * r : 8 * r + 8], cur[:], 0.0
                        )
                        cur = nxt

                # PE p-state warm-up during the selection window (junk)
                qw = psumw.tile([n_dom, D], f32)
                for _ in range(20):
                    nc.tensor.matmul(
                        qw[:], amat_t[:], xts[0][:, 0, :], start=True, stop=True
                    )

                # thr = v459 + 0.9*(v460 - v459); v460 = rank 52, v459 = rank 53
                nc.vector.tensor_tensor(
                    d1[:], mv[:, 51:52], mv[:, 52:53], op=Alu.subtract
                )
                nc.vector.scalar_tensor_tensor(
                    thr_t[:], d1[:], 0.9, mv[:, 52:53], op0=Alu.mult, op1=Alu.add
                )
                nc.vector.tensor_scalar(
                    mask3[:, 0, :],
                    im_all[:],
                    scalar1=thr_t[:, 0:1],
                    scalar2=None,
                    op0=Alu.is_gt,
                )
                if _dbg:
                    nc.gpsimd.dma_start(dbg_im[:], im_all[:])
                    dbgm = pp.tile([B, D], f32)
                    nc.vector.tensor_copy(dbgm[:], mask3[:, 0, :])
                    nc.gpsimd.dma_start(dbg_mask[:], dbgm[:])

            # ---- stage 3: psum = A^T@x + Pc^T@(x*mask); Act copies psum
            # -> fp16 SBUF; DMA fp16 -> out ----
            with (
                tc.tile_pool(name="xmp", bufs=3) as xmp,
                tc.tile_pool(name="otp", bufs=3) as otp,
                tc.tile_pool(name="qp", bufs=2, space="PSUM") as qp,
            ):
                for t0 in range(0, T_LOC, TG3):
                    xti = xts[t0 // TG1]
                    s0 = t0 % TG1
                    xs = xti[:, s0 : s0 + TG3, :]
                    xm = xmp.tile([B, TG3, D], f16, tag="xm")
                    nc.vector.tensor_tensor(
                        xm[:], xs, mask3[:].to_broadcast([B, TG3, D]), op=Alu.mult
                    )
                    q = qp.tile([n_dom, TG3, D], f32, tag="q")
                    for j in range(TG3):
                        nc.tensor.matmul(
                            q[:, j, :], amat_t[:], xs[:, j, :],
                            start=True, stop=False,
                        )
                        nc.tensor.matmul(
                            q[:, j, :], pmat_t[:], xm[:, j, :],
                            start=False, stop=True,
                        )
                    ot = otp.tile([n_dom, TG3, D], f16, tag="ot")
                    nc.scalar.copy(ot[:], q[:])
                    nc.sync.dma_start(out_sl[:, t0 : t0 + TG3, :], ot[:])
    nc.compile()
    return nc


def kernel(x, scenario_gradient, mixup_strength, scenario, partner_idx, is_dominant):
    global LAST_RESULT
    from concourse.bass_utils import run_bass_kernel_spmd

    x = np.ascontiguousarray(np.asarray(x, dtype=np.float32))
    dm = np.asarray(is_dominant, dtype=bool).ravel()
    dom = np.flatnonzero(dm)
    n_dom = int(dom.size)
    if n_dom == 0:
        return x.copy()

    g = np.ascontiguousarray(np.asarray(scenario_gradient, dtype=np.float32))
    m = np.asarray(mixup_strength, dtype=np.float32).ravel()
    p = np.asarray(partner_idx, dtype=np.int64).ravel()

    nc = _CACHE.get(n_dom)
    if nc is None:
        nc = _build(n_dom)
        _CACHE[n_dom] = nc

    # stationary matrices: amat gathers dominant rows; pmat = c*(P - I)
    j = np.arange(n_dom)
    amat = np.zeros((B, n_dom), dtype=np.float16)
    amat[dom, j] = 1.0
    c = (1.0 - m[dom]).astype(np.float32)
    pmat = np.zeros((B, n_dom), dtype=np.float32)
    np.add.at(pmat, (p[dom], j), c)
    np.add.at(pmat, (dom, j), -c)
    pmat16 = pmat.astype(np.float16)

    x16 = x.astype(np.float16)
    g16 = g.astype(np.float16)

    in_maps = []
    for ci in range(N_CORES):
        sl = slice(ci * T_LOC, (ci + 1) * T_LOC)
        in_maps.append(
            {
                "x_sl": np.ascontiguousarray(x16[:, sl, :]),
                "g_sl": np.ascontiguousarray(g16[:, sl, :]),
                "amat": amat,
                "pmat": pmat16,
            }
        )

    res = run_bass_kernel_spmd(nc, in_maps, core_ids=list(range(N_CORES)))
    LAST_RESULT = res

    out = x.copy()
    for ci in range(N_CORES):
        out[dom, ci * T_LOC : (ci + 1) * T_LOC, :] = res.results[ci]["out_sl"]
    return out


# revision 5
# speedup vs baseline: 2.3983x; 2.1747x over previous
"""CAFE-interpolation kernel for 8 Trainium2 NeuronCores.

Strategy: shard the T axis (1024 = 8 x 128) across cores. Every core holds a
T-slice of ALL 128 samples, so the sr[partner_idx] gather is core-local.

Math: with mask_b = (im_b > thr_b) in {0,1}^D and c_b = is_dominant_b*(1-m_b):

  out[b] = x[b] + c_b * ( mask[p_b] . x[p_b] - mask[b] . x[b] )

Only dominant rows differ from x, so the device returns just those rows
(packed via the matmul's stationary gather matrix); the host assembles
out = x.copy() and scatters the device rows in.

Per-core pipeline (inputs are fp16, host-converted; halves read traffic and
enables the DVE 2x 16-bit mode):

  stage 1: im_partial[b,d] = sum_{t in slice} g*x. DVE: fp16 product +
           pairwise tree-add (fp16), f32 accumulation across t-groups on
           GpSimd. x tiles stay resident in SBUF for stage 3 (16 MB).
  AllReduce im_partial [128, 512] f32 across 8 cores (~256 KB).
  stage 2: exact 52nd/53rd largest per row via 7 rounds of the DVE max-8
           instruction + match_replace (top-k extraction, 8 ranks/round);
           thr = v459 + 0.9*(v460-v459) exactly like jnp.quantile (the
           1/T mean scale cancels: mask is scale-invariant).
  stage 3: per t-group: xm = x * mask (DVE, fp16); PSUM accumulates
           A^T@x + Pc^T@xm where A packs dominant rows and Pc = c*(P - I);
           the f32 PSUM tile IS the output -> DMA straight to DRAM.

The same program works for every (partner_idx, is_dominant, mixup): the
metadata enters only through the amat/pmat input tensors; compile is keyed
only on n_dom.
"""

import os
import numpy as np

B, T, D = 128, 1024, 512
N_CORES = 8
T_LOC = T // N_CORES  # 128
TG1 = 8  # stage-1 t-steps per group (16 groups)
TG3 = 4  # stage-3 t-steps per group (32 groups); [n_dom, 4*512] f32 = 4 PSUM banks
NSEL = 7  # max-8 rounds: ranks 1..56 cover v460 (rank 52) and v459 (rank 53)

_CACHE: dict = {}
LAST_RESULT = None


def _build():
    import concourse.mybir as mybir
    import concourse.tile as tile
    from concourse import bacc

    f32 = mybir.dt.float32
    f16 = mybir.dt.float16
    Alu = mybir.AluOpType

    _dbg = os.environ.get("KBUILD_DEBUG") == "1"

    nc = bacc.Bacc(
        "TRN2", target_bir_lowering=False, debug=False, num_devices=N_CORES
    )
    x_sl = nc.dram_tensor("x_sl", [B, T_LOC, D], f16, kind="ExternalInput")
    g_sl = nc.dram_tensor("g_sl", [B, T_LOC, D], f16, kind="ExternalInput")
    # stationary matrices are padded to the full 128 columns (zeros past
    # n_dom) so every store DMA carries 128 partitions -- patterns with
    # fewer rows get pinned to a single DMA engine instead of the 16-way
    # split (measured: 71-row stores drained at 21 GB/s on one engine).
    amat_in = nc.dram_tensor("amat", [B, B], f16, kind="ExternalInput")
    pmat_in = nc.dram_tensor("pmat", [B, B], f16, kind="ExternalInput")
    out_sl = nc.dram_tensor("out_sl", [B, T_LOC, D], f16, kind="ExternalOutput")
    if _dbg:
        dbg_im = nc.dram_tensor("dbg_im", [B, D], f32, kind="ExternalOutput")
        dbg_mask = nc.dram_tensor("dbg_mask", [B, D], f32, kind="ExternalOutput")

    n_g1 = T_LOC // TG1

    with tile.TileContext(nc) as tc:
        with tc.tile_pool(name="persist", bufs=1) as pp:
            amat_t = pp.tile([B, B], f16)
            nc.sync.dma_start(amat_t[:], amat_in[:])
            pmat_t = pp.tile([B, B], f16)
            nc.sync.dma_start(pmat_t[:], pmat_in[:])

            # persistent x cache: 16 tiles of [128, 8, 512] fp16 (16 MB)
            xts = [pp.tile([B, TG1, D], f16, name=f"xc{i}") for i in range(n_g1)]

            imacc = pp.tile([B, D], f32)
            im_all = pp.tile([B, D], f32)
            sel_a = pp.tile([B, D], f32)
            sel_b = pp.tile([B, D], f32)
            mv = pp.tile([B, 8 * NSEL], f32)
            mask3 = pp.tile([B, 1, D], f16)
            thr_t = pp.tile([B, 1], f32)
            d1 = pp.tile([B, 1], f32)

            # ---- stage 1: im_partial = sum_t x*g ----
            with (
                tc.tile_pool(name="gld", bufs=2) as gld,
                tc.tile_pool(name="wk1", bufs=2) as wk1,
                tc.tile_pool(name="ccp", bufs=1, space="DRAM") as ccp,
            ):
                for i in range(n_g1):
                    t0 = i * TG1
                    nc.sync.dma_start(xts[i][:], x_sl[:, t0 : t0 + TG1, :])
                    gt = gld.tile([B, TG1, D], f16, tag="g1")
                    nc.sync.dma_start(gt[:], g_sl[:, t0 : t0 + TG1, :])
                    prod = wk1.tile([B, TG1, D], f16, tag="prod")
                    nc.vector.tensor_tensor(prod[:], xts[i][:], gt[:], op=Alu.mult)
                    l1 = wk1.tile([B, TG1 // 2, D], f16, tag="l1")
                    nc.vector.tensor_tensor(
                        l1[:], prod[:, 0 : TG1 // 2, :], prod[:, TG1 // 2 :, :],
                        op=Alu.add,
                    )
                    l2 = wk1.tile([B, TG1 // 4, D], f16, tag="l2")
                    nc.vector.tensor_tensor(
                        l2[:], l1[:, 0 : TG1 // 4, :], l1[:, TG1 // 4 :, :],
                        op=Alu.add,
                    )
                    if i == 0:
                        # f32 accumulator seeded directly on DVE
                        nc.vector.tensor_tensor(
                            imacc[:], l2[:, 0, :], l2[:, 1, :], op=Alu.add
                        )
                    else:
                        l3 = wk1.tile([B, D], f16, tag="l3")
                        nc.vector.tensor_tensor(
                            l3[:], l2[:, 0, :], l2[:, 1, :], op=Alu.add
                        )
                        # accumulate on GpSimd to keep DVE free
                        nc.gpsimd.tensor_tensor(
                            imacc[:], imacc[:], l3[:], op=Alu.add
                        )

                # ---- AllReduce the partial importance (no 1/T scale:
                # quantile mask is scale-invariant) ----
                cc_in_t = ccp.tile([B, D], f32, name="cc_in_t")
                cc_out_t = ccp.tile([B, D], f32, name="cc_out_t")
                nc.gpsimd.dma_start(cc_in_t[:], imacc[:])
                nc.gpsimd.collective_compute(
                    "AllReduce",
                    Alu.add,
                    replica_groups=[list(range(N_CORES))],
                    ins=[cc_in_t.opt()],
                    outs=[cc_out_t.opt()],
                )
                nc.gpsimd.dma_start(im_all[:], cc_out_t[:])

            # ---- stage 2: ranks 52/53 via 8-wide max extraction ----
            with tc.tile_pool(name="psumw", bufs=1, space="PSUM") as psumw:
                cur = im_all
                for r in range(NSEL):
                    nc.vector.max(mv[:, 8 * r : 8 * r + 8], cur[:])
                    if r < NSEL - 1:
                        nxt = sel_a if r % 2 == 0 else sel_b
                        nc.vector.match_replace(
                            nxt[:], mv[:, 8 * r : 8 * r + 8], cur[:], 0.0
                        )
                        cur = nxt

                # PE p-state warm-up during the selection window (junk)
                qw = psumw.tile([B, D], f32)
                for _ in range(20):
                    nc.tensor.matmul(
                        qw[:], amat_t[:], xts[0][:, 0, :], start=True, stop=True
                    )

                # thr = v459 + 0.9*(v460 - v459); v460 = rank 52, v459 = rank 53
                nc.vector.tensor_tensor(
                    d1[:], mv[:, 51:52], mv[:, 52:53], op=Alu.subtract
                )
                nc.vector.scalar_tensor_tensor(
                    thr_t[:], d1[:], 0.9, mv[:, 52:53], op0=Alu.mult, op1=Alu.add
                )
                nc.vector.tensor_scalar(
                    mask3[:, 0, :],
                    im_all[:],
                    scalar1=thr_t[:, 0:1],
                    scalar2=None,
                    op0=Alu.is_gt,
                )
                if _dbg:
                    nc.gpsimd.dma_start(dbg_im[:], im_all[:])
                    dbgm = pp.tile([B, D], f32)
                    nc.vector.tensor_copy(dbgm[:], mask3[:, 0, :])
                    nc.gpsimd.dma_start(dbg_mask[:], dbgm[:])

            # ---- stage 3: psum = A^T@x + Pc^T@(x*mask); Act copies psum
            # -> fp16 SBUF; DMA fp16 -> out ----
            with (
                tc.tile_pool(name="xmp", bufs=3) as xmp,
                tc.tile_pool(name="otp", bufs=3) as otp,
                tc.tile_pool(name="qp", bufs=2, space="PSUM") as qp,
            ):
                for t0 in range(0, T_LOC, TG3):
                    xti = xts[t0 // TG1]
                    s0 = t0 % TG1
                    xs = xti[:, s0 : s0 + TG3, :]
                    xm = xmp.tile([B, TG3, D], f16, tag="xm")
                    nc.vector.tensor_tensor(
                        xm[:], xs, mask3[:].to_broadcast([B, TG3, D]), op=Alu.mult
                    )
                    q = qp.tile([B, TG3, D], f32, tag="q")
                    for j in range(TG3):
                        nc.tensor.matmul(
                            q[:, j, :], amat_t[:], xs[:, j, :],
                            start=True, stop=False,
                        )
                        nc.tensor.matmul(
                            q[:, j, :], pmat_t[:], xm[:, j, :],
                            start=False, stop=True,
                        )
                    ot = otp.tile([B, TG3, D], f16, tag="ot")
                    nc.scalar.copy(ot[:], q[:])
                    nc.sync.dma_start(out_sl[:, t0 : t0 + TG3, :], ot[:])
    nc.compile()
    return nc


def kernel(x, scenario_gradient, mixup_strength, scenario, partner_idx, is_dominant):
    global LAST_RESULT
    from concourse.bass_utils import run_bass_kernel_spmd

    x = np.ascontiguousarray(np.asarray(x, dtype=np.float32))
    dm = np.asarray(is_dominant, dtype=bool).ravel()
    dom = np.flatnonzero(dm)
    n_dom = int(dom.size)
    if n_dom == 0:
        return x.copy()

    g = np.ascontiguousarray(np.asarray(scenario_gradient, dtype=np.float32))
    m = np.asarray(mixup_strength, dtype=np.float32).ravel()
    p = np.asarray(partner_idx, dtype=np.int64).ravel()

    nc = _CACHE.get("main")
    if nc is None:
        nc = _build()
        _CACHE["main"] = nc

    # stationary matrices: amat gathers dominant rows; pmat = c*(P - I);
    # columns n_dom..127 stay zero (output rows ignored by the host)
    j = np.arange(n_dom)
    amat = np.zeros((B, B), dtype=np.float16)
    amat[dom, j] = 1.0
    c = (1.0 - m[dom]).astype(np.float32)
    pmat = np.zeros((B, B), dtype=np.float32)
    np.add.at(pmat, (p[dom], j), c)
    np.add.at(pmat, (dom, j), -c)
    pmat16 = pmat.astype(np.float16)

    x16 = x.astype(np.float16)
    g16 = g.astype(np.float16)

    in_maps = []
    for ci in range(N_CORES):
        sl = slice(ci * T_LOC, (ci + 1) * T_LOC)
        in_maps.append(
            {
                "x_sl": np.ascontiguousarray(x16[:, sl, :]),
                "g_sl": np.ascontiguousarray(g16[:, sl, :]),
                "amat": amat,
                "pmat": pmat16,
            }
        )

    res = run_bass_kernel_spmd(nc, in_maps, core_ids=list(range(N_CORES)))
    LAST_RESULT = res

    out = x.copy()
    for ci in range(N_CORES):
        out[dom, ci * T_LOC : (ci + 1) * T_LOC, :] = res.results[ci]["out_sl"][:n_dom]
    return out
